# revision 2
# baseline (speedup 1.0000x reference)
"""Trainium2 Bass kernel for 3-layer heterogeneous GraphSAGE (EntityGraphNN).

8 NeuronCores, SPMD single program:
  - Destination-node sharding: each core owns 1/8 of each node type's
    128-row tiles. Edges routed to the core owning their dst.
  - h[src] row gathers (256B) via gpsimd dma_gather over 4 SWDGE queues.
    Sources banked by 32768 rows (int16 index limit). One gather per
    (stripe-of-4-dst-tiles, src-bank): fixed quota Q slots per (tile, bank)
    cell + one 128-slot overflow chunk (trailing -1 idxs are skipped).
  - Scatter-add via one-hot matmul: oh[e,d] = (dstl[e]==d) on DVE, PSUM
    agg_T[64, 512] accumulates G^T @ oh per stripe.
  - mean = agg * invcnt (host-precomputed), z = mean@Wl + h_dst@Wr + b and
    its transpose z_T by the swapped matmul pair; ReLU; h stored row-major
    (gather table) and transposed (next layer's root term).
  - Input projections replicated on every core (no layer-0 AllGather).
  - AllGather of h after layers 0 and 1; layer 2 computes only cheval
    logits.
"""
import numpy as np

HID = 64
P = 128
BANK = 32768
N_CORES = 8
STRIPE_T = 4
OVF = 128

_ETYPES = {
    "rev_part": ("c", "r"),
    "monte": ("c", "j"),
    "part": ("r", "c"),
    "rev_monte": ("j", "c"),
}
_DST_ETYPES = {"c": ["rev_part", "monte"], "r": ["part"], "j": ["rev_monte"]}


def _ceil(a, b):
    return (a + b - 1) // b


def _prep_edge_type(ei, n_src_pad, tpc, q):
    src = ei[0].astype(np.int64)
    dst = ei[1].astype(np.int64)
    nbanks = _ceil(n_src_pad, BANK)
    shard = tpc * P
    core = dst // shard
    tile = (dst % shard) // P
    bank = src // BANK

    stripes = []
    t0 = 0
    while t0 < tpc:
        stripes.append((t0, min(STRIPE_T, tpc - t0)))
        t0 += STRIPE_T

    order = np.lexsort((src, bank, tile, core))
    so, do_ = src[order], dst[order]
    co = core[order]
    to = tile[order]
    bo = bank[order]
    key = (co * tpc + to) * nbanks + bo
    ncell = N_CORES * tpc * nbanks
    cnts = np.bincount(key, minlength=ncell).reshape(N_CORES, tpc, nbanks)
    starts = np.zeros(ncell + 1, np.int64)
    np.cumsum(cnts.reshape(-1), out=starts[1:])

    runs = []
    total_slots = 0
    for si, (ts, nt) in enumerate(stripes):
        for b in range(nbanks):
            oc = 0
            for c in range(N_CORES):
                ov = int(np.maximum(cnts[c, ts:ts + nt, b] - q, 0).sum())
                oc = max(oc, ov)
            if oc > OVF:
                return None
            n_idx = nt * q + OVF
            n_reg = n_idx
            runs.append((si, b, nt, n_idx, n_reg))
            total_slots += n_idx
    n_chunks = total_slots // P
    w_tot = total_slots // 16

    idx_all = np.zeros((N_CORES, 16, w_tot), np.int16)
    dstl_all = np.full((N_CORES, P, n_chunks), -1.0, np.float32)

    wofs = 0
    cofs = 0
    run_offsets = []
    for (si, b, nt, n_idx, n_reg) in runs:
        ts = stripes[si][0]
        for c in range(N_CORES):
            idxs = np.full(n_idx, -1, np.int64)
            dls = np.full(n_idx, -1.0, np.float32)
            ovs_i, ovs_d = [], []
            for j, t in enumerate(range(ts, ts + nt)):
                k = (c * tpc + t) * nbanks + b
                s0, s1 = int(starts[k]), int(starts[k + 1])
                cell_src = so[s0:s1] - b * BANK
                cell_dst = do_[s0:s1] - (c * shard + t * P)
                take = min(s1 - s0, q)
                idxs[j * q:j * q + take] = cell_src[:take]
                idxs[j * q + take:(j + 1) * q] = 0
                dls[j * q:j * q + take] = cell_dst[:take]
                if s1 - s0 > take:
                    ovs_i.extend(cell_src[take:].tolist())
                    ovs_d.extend((cell_dst[take:] + j * P).tolist())
            no = len(ovs_i)
            base = nt * q
            if no:
                idxs[base:base + no] = ovs_i
                dls[base:base + no] = ovs_d
            idxs[base + no:n_idx] = 0
            idx_all[c, :, wofs:wofs + n_idx // 16] = idxs.reshape(n_idx // 16, 16).T
            ch = n_idx // P
            dstl_all[c, :, cofs:cofs + ch] = dls.reshape(ch, P).T
        run_offsets.append((wofs, cofs, n_idx // P))
        wofs += n_idx // 16
        cofs += n_idx // P

    idx_rep = np.tile(idx_all, (1, 8, 1))
    return dict(runs=runs, run_offsets=run_offsets, stripes=stripes,
                idx=idx_rep, dstl=dstl_all, nbanks=nbanks, q=q,
                n_chunks=n_chunks, w_tot=w_tot)


def _prep_type(ei, n_src_pad, tpc, q0):
    q = q0
    while True:
        r = _prep_edge_type(ei, n_src_pad, tpc, q)
        if r is not None:
            return r
        q += P


def _invcnt(ei, n_dst_pad, tpc):
    cnt = np.bincount(ei[1].astype(np.int64), minlength=n_dst_pad).astype(np.float32)
    inv = 1.0 / np.maximum(cnt, 1.0)
    return inv.reshape(N_CORES, tpc, P)


def kernel(**inputs):
    import concourse.bass as bass
    import concourse.mybir as mybir
    import concourse.tile as tile
    import concourse.bacc as bacc
    import jax
    from jax.sharding import Mesh, PartitionSpec, NamedSharding
    from jax.experimental.shard_map import shard_map
    from concourse.bass2jax import (_bass_exec_p, partition_id_tensor,
                                    install_neuronx_cc_hook)

    f32 = mybir.dt.float32
    x_np = {"c": np.asarray(inputs["x_cheval"], np.float32),
            "j": np.asarray(inputs["x_jockey"], np.float32),
            "r": np.asarray(inputs["x_course"], np.float32)}
    NC = x_np["c"].shape[0]

    tpc = {k: _ceil(x_np[k].shape[0], P * N_CORES) for k in x_np}
    npad = {k: tpc[k] * P * N_CORES for k in tpc}
    # +1 input row of ones folds the input-proj bias in
    din = {k: x_np[k].shape[1] + 1 for k in x_np}

    xT = {}
    for k in x_np:
        xt = np.zeros((din[k], npad[k]), np.float32)
        xt[:-1, :x_np[k].shape[0]] = x_np[k].T
        xt[-1, :] = 1.0
        xT[k] = xt

    w_in_np = {}
    for k, nm in (("c", "cheval"), ("j", "jockey"), ("r", "course")):
        w = np.asarray(inputs[f"w_in_{nm}"], np.float32)
        b = np.asarray(inputs[f"b_in_{nm}"], np.float32)
        w_in_np[k] = np.concatenate([w, b.reshape(1, HID)], axis=0)

    w_cls = np.asarray(inputs["w_cls"], np.float32)
    b_cls = float(np.asarray(inputs["b_cls"]).reshape(-1)[0])
    eis = {k: np.asarray(inputs["ei_" + k]) for k in _ETYPES}
    NLAYERS = np.asarray(inputs["wl_part"]).shape[0]

    prep, iv = {}, {}
    for et, (dk, sk) in _ETYPES.items():
        lam = eis[et].shape[1] / (npad[dk] / P)
        nb = _ceil(npad[sk], BANK)
        q0 = max(P, _ceil(int(lam / nb) + 1, P) * P)
        prep[et] = _prep_type(eis[et], npad[sk], tpc[dk], q0)
        iv[et] = _invcnt(eis[et], npad[dk], tpc[dk])

    WL = {et: np.asarray(inputs["wl_" + et], np.float32) for et in _ETYPES}
    BL = {et: np.asarray(inputs["bl_" + et], np.float32) for et in _ETYPES}
    WR = {et: np.asarray(inputs["wr_" + et], np.float32) for et in _ETYPES}
    WRc = {dk: sum(WR[et] for et in _DST_ETYPES[dk]) for dk in _DST_ETYPES}
    Bc = {dk: sum(BL[et] for et in _DST_ETYPES[dk]) for dk in _DST_ETYPES}
    bias_nonzero = {dk: bool(np.any(Bc[dk])) for dk in _DST_ETYPES}

    nc = bacc.Bacc(None, num_swdge_queues=4)

    din_t = {k: nc.declare_dram_parameter(f"xT_{k}", [din[k], npad[k]], f32, False)
             for k in tpc}
    xown_t = {k: nc.declare_dram_parameter(f"xo_{k}", [din[k], tpc[k] * P], f32, False)
              for k in tpc}
    win_t = {k: nc.declare_dram_parameter(f"win_{k}", [din[k], HID], f32, False)
             for k in tpc}
    idx_t = {et: nc.declare_dram_parameter(f"idx_{et}", [P, prep[et]["w_tot"]],
                                           mybir.dt.int16, False) for et in _ETYPES}
    dstl_t = {et: nc.declare_dram_parameter(f"dstl_{et}", [P, prep[et]["n_chunks"]],
                                            f32, False) for et in _ETYPES}
    ivc_t = {et: nc.declare_dram_parameter(f"ivc_{et}", [P, tpc[_ETYPES[et][0]]],
                                           f32, False) for et in _ETYPES}
    wl_t = {et: nc.declare_dram_parameter(f"wl_{et}", [NLAYERS, HID, HID], f32, False)
            for et in _ETYPES}
    wrc_t = {dk: nc.declare_dram_parameter(f"wrc_{dk}", [NLAYERS, HID, HID], f32,
                                           False) for dk in _DST_ETYPES}
    bc_t = {dk: nc.declare_dram_parameter(f"bc_{dk}", [NLAYERS, 1, HID], f32, False)
            for dk in _DST_ETYPES}
    iota128_t = nc.declare_dram_parameter("iota128", [P, P], f32, False)
    iota512_t = nc.declare_dram_parameter("iota512", [P, 512], f32, False)
    wclsr_t = nc.declare_dram_parameter("wclsr", [P, STRIPE_T * HID], f32, False)
    out_t = nc.declare_dram_parameter("out", [tpc["c"] * P, 1], f32, True)


    with tile.TileContext(nc) as tc:
        with (
            tc.tile_pool(name="dram", bufs=1, space="DRAM") as dram,
            tc.tile_pool(name="wpool", bufs=1) as wpool,
            tc.tile_pool(name="gpool", bufs=3) as gpool,
            tc.tile_pool(name="ohpool", bufs=4) as ohpool,
            tc.tile_pool(name="pool", bufs=2) as pool,
            tc.tile_pool(name="psum", bufs=2, space="PSUM") as psum,
        ):
            h_full = {}
            hT_loc = {}
            shard_buf = {}
            ag_out = {}
            for k in tpc:
                h0 = nc.dram_tensor(f"h0{k}", [npad[k], HID], f32)
                ag_out[k] = [nc.dram_tensor(f"ag{k}{l}", [npad[k], HID], f32,
                                            addr_space="Shared")
                             for l in range(2)]
                h_full[k] = [h0, ag_out[k][0], ag_out[k][1]]
                hT_loc[k] = [nc.dram_tensor(f"hT{k}{l}", [HID, tpc[k] * P], f32)
                             for l in range(3)]
                shard_buf[k] = [nc.dram_tensor(f"sh{k}{l}", [tpc[k] * P, HID], f32)
                                for l in range(2)]

            from concourse.masks import make_identity
            ident = wpool.tile([P, P], f32)
            make_identity(nc, ident[:])
            iota128 = wpool.tile([P, P], f32)
            nc.sync.dma_start(iota128[:], iota128_t[:])
            iota512 = wpool.tile([P, 512], f32)
            nc.sync.dma_start(iota512[:], iota512_t[:])
            ones1 = wpool.tile([1, P], f32)
            nc.gpsimd.memset(ones1[:], 1.0)
            wclsr2 = wpool.tile([P, STRIPE_T * HID], f32)
            nc.sync.dma_start(wclsr2[:], wclsr_t[:])
            win_sb = {}
            for k in tpc:
                win_sb[k] = wpool.tile([din[k], HID], f32, tag=f"win{k}", name=f"win{k}")
                nc.sync.dma_start(win_sb[k][:], win_t[k][:])
            wl_sb, wrc_sb, bc_sb = {}, {}, {}
            for et in _ETYPES:
                for l in range(NLAYERS):
                    wl_sb[(et, l)] = wpool.tile([HID, HID], f32, tag=f"wl{et}{l}", name=f"wl{et}{l}")
                    nc.sync.dma_start(wl_sb[(et, l)][:], wl_t[et][l])
            for dk in _DST_ETYPES:
                for l in range(NLAYERS):
                    wrc_sb[(dk, l)] = wpool.tile([HID, HID], f32, tag=f"wrc{dk}{l}", name=f"wrc{dk}{l}")
                    nc.sync.dma_start(wrc_sb[(dk, l)][:], wrc_t[dk][l])
                    bc_sb[(dk, l)] = wpool.tile([1, HID], f32, tag=f"bc{dk}{l}", name=f"bc{dk}{l}")
                    nc.sync.dma_start(bc_sb[(dk, l)][:], bc_t[dk][l])

            # ---- input projection (full, replicated) ----
            GB = 4
            for k in tpc:
                ntile = npad[k] // P
                for g0 in range(0, ntile, GB):
                    gn = min(GB, ntile - g0)
                    xt = gpool.tile([din[k], GB * P], f32, tag=f"xt{k}")
                    nc.sync.dma_start(xt[:, :gn * P],
                                      din_t[k][:, g0 * P:(g0 + gn) * P])
                    zp = psum.tile([P, 256], f32, space="PSUM", tag="zmr")
                    for j in range(gn):
                        nc.tensor.matmul(
                            out=zp[:, j * HID:(j + 1) * HID],
                            lhsT=xt[:, j * P:(j + 1) * P],
                            rhs=win_sb[k][:], start=(j == 0), stop=(j == gn - 1),
                            skip_group_check=True)
                    zr = pool.tile([P, STRIPE_T * HID], f32, tag="zr")
                    nc.vector.tensor_scalar(
                        out=zr[:, :gn * HID], in0=zp[:, :gn * HID],
                        scalar1=0.0, scalar2=None, op0=mybir.AluOpType.max)
                    for j in range(gn):
                        nc.sync.dma_start(
                            h_full[k][0][:][(g0 + j) * P:(g0 + j + 1) * P, :],
                            zr[:, j * HID:(j + 1) * HID])
                for g0 in range(0, tpc[k], GB):
                    gn = min(GB, tpc[k] - g0)
                    xt = gpool.tile([din[k], GB * P], f32, tag=f"xt{k}")
                    nc.sync.dma_start(xt[:, :gn * P],
                                      xown_t[k][:, g0 * P:(g0 + gn) * P])
                    ztp = psum.tile([HID, 512], f32, space="PSUM", tag="ztp")
                    for j in range(gn):
                        nc.tensor.matmul(
                            out=ztp[:, j * P:(j + 1) * P],
                            lhsT=win_sb[k][:],
                            rhs=xt[:, j * P:(j + 1) * P], start=(j == 0),
                            stop=(j == gn - 1), skip_group_check=True)
                    ztr = pool.tile([HID, STRIPE_T * P], f32, tag="ztr")
                    nc.vector.tensor_scalar(
                        out=ztr[:, :gn * P], in0=ztp[:, :gn * P],
                        scalar1=0.0, scalar2=None, op0=mybir.AluOpType.max)
                    nc.sync.dma_start(hT_loc[k][0][:, g0 * P:(g0 + gn) * P],
                                      ztr[:, :gn * P])

            # memset gather buffers once (stale-read safety under masks)
            g_chunks = {}
            for et in _ETYPES:
                mx = max(r[3] for r in prep[et]["runs"]) // P
                g_chunks[et] = mx
                for _ in range(3):
                    t_ = gpool.tile([P, mx * HID], f32, tag=f"g{et}")
                    nc.gpsimd.memset(t_[:], 0.0)

            qrot = [0]

            def do_layer(l):
                last = (l == NLAYERS - 1)
                dks = ["c"] if last else ["j", "r", "c"]
                for dk in dks:
                    ets = _DST_ETYPES[dk]
                    stripes = prep[ets[0]]["stripes"]
                    for si, (ts, nt) in enumerate(stripes):
                        ivs = {}
                        for et in ets:
                            ivt = pool.tile([P, STRIPE_T], f32, tag=f"iv{et}",
                                            name=f"iv{et}")
                            nc.sync.dma_start(ivt[:, :nt],
                                              ivc_t[et][:, ts:ts + nt])
                            ivs[et] = ivt
                        aggs = {}
                        for eti, et in enumerate(ets):
                            pr = prep[et]
                            agg = psum.tile([HID, 512], f32, space="PSUM",
                                            tag=f"agg{eti}")
                            aggs[et] = agg
                            rlist = [(i, r) for i, r in enumerate(pr["runs"])
                                     if r[0] == si]
                            n_r = len(rlist)
                            q = pr["q"]
                            cpt = q // P
                            for ri, (gi, (_, b, nt_, n_idx, n_reg)) in enumerate(rlist):
                                wofs, cofs, ch = pr["run_offsets"][gi]
                                w = n_idx // 16
                                it = gpool.tile([P, w], mybir.dt.int16,
                                                tag=f"it{et}")
                                nc.sync.dma_start(it[:], idx_t[et][:, wofs:wofs + w])
                                dt_ = gpool.tile([P, ch], f32, tag=f"dt{et}")
                                nc.sync.dma_start(dt_[:],
                                                  dstl_t[et][:, cofs:cofs + ch])
                                gt = gpool.tile([P, g_chunks[et] * HID], f32,
                                                tag=f"g{et}")
                                sk = _ETYPES[et][1]
                                tab = h_full[sk][l]
                                b_hi = min((b + 1) * BANK, npad[sk])
                                nc.gpsimd.dma_gather(
                                    out_ap=gt[:, :ch * HID].rearrange(
                                        "p (c f) -> p c f", f=HID),
                                    in_ap=tab[:][b * BANK:b_hi, :],
                                    idxs_ap=it[:, :],
                                    num_idxs=n_idx, num_idxs_reg=n_reg,
                                    elem_size=HID, single_packet=False,
                                    queue_num=qrot[0] % 4)
                                qrot[0] += 1
                                for c in range(ch):
                                    is_ovf = (c >= nt_ * cpt)
                                    wdt = nt_ * P if is_ovf else P
                                    io = iota512 if is_ovf else iota128
                                    oh = ohpool.tile([P, 512], f32, tag="oh")
                                    nc.vector.tensor_tensor(
                                        out=oh[:, :wdt],
                                        in0=dt_[:, c:c + 1].to_broadcast([P, wdt]),
                                        in1=io[:, :wdt],
                                        op=mybir.AluOpType.is_equal)
                                    if is_ovf:
                                        for tt in range(nt_):
                                            lastmm = (ri == n_r - 1
                                                      and c == ch - 1
                                                      and tt == nt_ - 1)
                                            nc.tensor.matmul(
                                                out=agg[:, tt * P:(tt + 1) * P],
                                                lhsT=gt[:, c * HID:(c + 1) * HID],
                                                rhs=oh[:, tt * P:(tt + 1) * P],
                                                start=False, stop=lastmm,
                                                skip_group_check=True)
                                    else:
                                        tt = c // cpt
                                        o0, o1 = tt * P, (tt + 1) * P
                                        first = (ri == 0 and c == 0)
                                        nc.tensor.matmul(
                                            out=agg[:, o0:o1],
                                            lhsT=gt[:, c * HID:(c + 1) * HID],
                                            rhs=oh[:, :wdt],
                                            start=first, stop=False,
                                            skip_group_check=True)
                        aggsb = {}
                        for et in ets:
                            a = pool.tile([HID, 512], f32, tag=f"aggsb{et}",
                                          name=f"aggsb{et}")
                            nc.vector.tensor_copy(a[:, :nt * P],
                                                  aggs[et][:, :nt * P])
                            aggsb[et] = a
                        zsb = pool.tile([P, STRIPE_T * HID], f32, tag="zsb")
                        for j in range(nt):
                            t = ts + j
                            zmr = psum.tile([P, 256], f32, space="PSUM",
                                            tag="zmr")
                            for ei_, et in enumerate(ets):
                                nc.tensor.matmul(
                                    out=zmr[:, ei_ * HID:(ei_ + 1) * HID],
                                    lhsT=aggsb[et][:, j * P:(j + 1) * P],
                                    rhs=wl_sb[(et, l)][:],
                                    start=(ei_ == 0), stop=False,
                                    skip_group_check=True)
                            hT = pool.tile([HID, P], f32, tag="hTt")
                            nc.sync.dma_start(
                                hT[:], hT_loc[dk][l][:][:, t * P:(t + 1) * P])
                            ro = 2 * HID
                            nc.tensor.matmul(out=zmr[:, ro:ro + HID],
                                             lhsT=hT[:],
                                             rhs=wrc_sb[(dk, l)][:],
                                             start=False,
                                             stop=not bias_nonzero[dk],
                                             skip_group_check=True)
                            if bias_nonzero[dk]:
                                nc.tensor.matmul(out=zmr[:, ro:ro + HID],
                                                 lhsT=ones1[:],
                                                 rhs=bc_sb[(dk, l)][:],
                                                 start=False, stop=True,
                                                 skip_group_check=True)
                            # z = sum_et ivc_et * zm_et + zroot, then relu
                            zrt = pool.tile([P, HID], f32, tag="zrt")
                            nc.vector.tensor_copy(zrt[:], zmr[:, ro:ro + HID])
                            tmp = pool.tile([P, HID], f32, tag="ztmp")
                            nc.vector.scalar_tensor_tensor(
                                out=tmp[:],
                                in0=zmr[:, 0:HID],
                                scalar=ivs[ets[0]][:, j:j + 1],
                                in1=zrt[:],
                                op0=mybir.AluOpType.mult,
                                op1=mybir.AluOpType.add)
                            if len(ets) > 1:
                                nc.vector.scalar_tensor_tensor(
                                    out=tmp[:],
                                    in0=zmr[:, HID:2 * HID],
                                    scalar=ivs[ets[1]][:, j:j + 1],
                                    in1=tmp[:],
                                    op0=mybir.AluOpType.mult,
                                    op1=mybir.AluOpType.add)
                            nc.vector.tensor_scalar(
                                out=zsb[:, j * HID:(j + 1) * HID], in0=tmp[:],
                                scalar1=0.0, scalar2=None,
                                op0=mybir.AluOpType.max)
                            if not last:
                                ztp = psum.tile([HID, 512], f32, space="PSUM",
                                                tag="ztp")
                                nc.tensor.transpose(
                                    out=ztp[:, :P],
                                    in_=zsb[:, j * HID:(j + 1) * HID],
                                    identity=ident[:])
                                ztr2 = pool.tile([HID, P], f32, tag="ztr2")
                                nc.vector.tensor_copy(ztr2[:], ztp[:, :P])
                                nc.sync.dma_start(
                                    hT_loc[dk][l + 1][:][:, t * P:(t + 1) * P],
                                    ztr2[:])
                        if not last:
                            for j in range(nt):
                                nc.sync.dma_start(
                                    shard_buf[dk][l][:][(ts + j) * P:(ts + j + 1) * P, :],
                                    zsb[:, j * HID:(j + 1) * HID])
                        else:
                            tmp2 = pool.tile([P, STRIPE_T * HID], f32, tag="ctmp")
                            nc.vector.tensor_tensor(
                                out=tmp2[:, :nt * HID], in0=zsb[:, :nt * HID],
                                in1=wclsr2[:, :nt * HID],
                                op=mybir.AluOpType.mult)
                            ot = pool.tile([P, STRIPE_T], f32, tag="otile")
                            nc.vector.tensor_reduce(
                                out=ot[:, :nt],
                                in_=tmp2[:, :nt * HID].rearrange(
                                    "p (t f) -> p t f", f=HID),
                                axis=mybir.AxisListType.X,
                                op=mybir.AluOpType.add)
                            if b_cls != 0.0:
                                nc.vector.tensor_scalar(
                                    out=ot[:, :nt], in0=ot[:, :nt],
                                    scalar1=b_cls, scalar2=None,
                                    op0=mybir.AluOpType.add)
                            oap = out_t[:].rearrange("(t p) o -> p t o", p=P)
                            nc.sync.dma_start(oap[:, ts:ts + nt, 0], ot[:, :nt])
                    if not last:
                        tc.strict_bb_all_engine_barrier()
                        nc.gpsimd.collective_compute(
                            "AllGather", mybir.AluOpType.bypass,
                            ins=[shard_buf[dk][l][:]],
                            outs=[ag_out[dk][l][:]],
                            replica_groups=[list(range(N_CORES))])

            for l in range(NLAYERS):
                tc.strict_bb_all_engine_barrier()
                do_layer(l)

    nc.finalize()

    iota128_v = np.broadcast_to(np.arange(P, dtype=np.float32), (P, P)).copy()
    iota512_v = np.broadcast_to(np.arange(512, dtype=np.float32), (P, 512)).copy()
    wclsr_v = np.tile(w_cls.reshape(1, HID), (P, STRIPE_T)).astype(np.float32)

    in_maps = []
    for c in range(N_CORES):
        m = {}
        for k in tpc:
            sh = tpc[k] * P
            m[f"xT_{k}"] = xT[k]
            m[f"xo_{k}"] = np.ascontiguousarray(xT[k][:, c * sh:(c + 1) * sh])
            m[f"win_{k}"] = w_in_np[k]
        for et in _ETYPES:
            m[f"idx_{et}"] = prep[et]["idx"][c]
            m[f"dstl_{et}"] = prep[et]["dstl"][c]
            m[f"ivc_{et}"] = np.ascontiguousarray(iv[et][c].T)
            m[f"wl_{et}"] = WL[et]
        for dk in _DST_ETYPES:
            m[f"wrc_{dk}"] = np.asarray(WRc[dk], np.float32)
            m[f"bc_{dk}"] = np.asarray(Bc[dk], np.float32).reshape(NLAYERS, 1, HID)
        m["iota128"] = iota128_v
        m["iota512"] = iota512_v
        m["wclsr"] = wclsr_v
        in_maps.append(m)

    install_neuronx_cc_hook()
    partition_name = nc.partition_id_tensor.name if nc.partition_id_tensor else None
    in_names, out_names, out_avals, zero_outs = [], [], [], []
    for alloc in nc.m.functions[0].allocations:
        if not isinstance(alloc, mybir.MemoryLocationSet):
            continue
        name = alloc.memorylocations[0].name
        if alloc.kind == "ExternalInput":
            if name != partition_name:
                in_names.append(name)
        elif alloc.kind == "ExternalOutput":
            out_names.append(name)
            shape = tuple(alloc.tensor_shape)
            dtype = mybir.dt.np(alloc.dtype)
            out_avals.append(jax.core.ShapedArray(shape, dtype))
            zero_outs.append(np.zeros(shape, dtype))
    n_params = len(in_names)
    all_in = list(in_names) + list(out_names)
    if partition_name is not None:
        all_in.append(partition_name)

    def _body(*args):
        operands = list(args)
        if partition_name is not None:
            operands.append(partition_id_tensor())
        outs = _bass_exec_p.bind(
            *operands, out_avals=tuple(out_avals), in_names=tuple(all_in),
            out_names=tuple(out_names), lowering_input_output_aliases=(),
            sim_require_finite=False, sim_require_nnan=False, nc=nc)
        return tuple(outs)

    devices = jax.devices()[:N_CORES]
    mesh = Mesh(np.asarray(devices), ("core",))
    specs = (PartitionSpec("core"),)
    sharded = jax.jit(
        shard_map(_body, mesh=mesh, in_specs=specs * (n_params + len(out_names)),
                  out_specs=specs * len(out_names), check_rep=False),
        keep_unused=True)
    per_core = [[np.asarray(m[n]) for n in in_names] for m in in_maps]
    concat_in = [np.concatenate([per_core[c][i] for c in range(N_CORES)], axis=0)
                 for i in range(n_params)]
    concat_zero = [np.zeros((N_CORES * z.shape[0], *z.shape[1:]), z.dtype)
                   for z in zero_outs]
    shd = NamedSharding(mesh, PartitionSpec("core"))
    dev_in = [
        jax.make_array_from_callback(a.shape, shd, lambda idx, a=a: a[idx])
        for a in concat_in + concat_zero
    ]
    outs = sharded(*dev_in)
    jax.block_until_ready(outs)
    import os as _os
    if _os.environ.get("BASS_KERNEL_TIME"):
        import time as _time
        times = []
        for _ in range(int(_os.environ.get("BASS_KERNEL_REPS", "8"))):
            t0 = _time.perf_counter()
            outs2 = sharded(*dev_in)
            jax.block_until_ready(outs2)
            times.append(_time.perf_counter() - t0)
        print(f"HW exec time: {min(times) * 1e9:.0f} ns")
        print(f"exec times (s): {[f'{t:.4f}' for t in times]}")
    oi = out_names.index("out")
    full = np.asarray(outs[oi]).reshape(N_CORES * tpc["c"] * P, 1)
    return full[:NC, :].astype(np.float32)



# revision 7
# speedup vs baseline: 1.9503x; 1.9503x over previous
"""Trainium2 Bass kernel for 3-layer heterogeneous GraphSAGE (EntityGraphNN).

8 NeuronCores, SPMD single program:
  - Destination-node sharding: each core owns 1/8 of each node type's
    128-row tiles. Edges routed to the core owning their dst.
  - h[src] row gathers (256B) via gpsimd dma_gather over 4 SWDGE queues.
    Sources banked by 32768 rows (int16 index limit). One gather per
    (stripe-of-4-dst-tiles, src-bank): fixed quota Q slots per (tile, bank)
    cell + one 128-slot overflow chunk (trailing -1 idxs are skipped).
  - Scatter-add via one-hot matmul: oh[e,d] = (dstl[e]==d) on DVE, PSUM
    agg_T[64, 512] accumulates G^T @ oh per stripe.
  - mean = agg * invcnt (host-precomputed), z = mean@Wl + h_dst@Wr + b and
    its transpose z_T by the swapped matmul pair; ReLU; h stored row-major
    (gather table) and transposed (next layer's root term).
  - Input projections replicated on every core (no layer-0 AllGather).
  - AllGather of h after layer 0 (all types) and layer 1 (r, j only --
    layer 2 computes only cheval logits and never gathers from c).
  - idx+dstl packed in one int16 load per run; invcnt preloaded once;
    per-stripe batched DMAs for shard/hT writes and hT reads.
"""
import numpy as np

HID = 64
P = 128
BANK = 32768
N_CORES = 8
STRIPE_T = 4
OVF = 128

_ETYPES = {
    "rev_part": ("c", "r"),
    "monte": ("c", "j"),
    "part": ("r", "c"),
    "rev_monte": ("j", "c"),
}
_DST_ETYPES = {"c": ["rev_part", "monte"], "r": ["part"], "j": ["rev_monte"]}


def _ceil(a, b):
    return (a + b - 1) // b


def _prep_edge_type(ei, n_src_pad, tpc, q):
    src = ei[0].astype(np.int64)
    dst = ei[1].astype(np.int64)
    nbanks = _ceil(n_src_pad, BANK)
    shard = tpc * P
    core = dst // shard
    tile = (dst % shard) // P
    bank = src // BANK

    stripes = []
    t0 = 0
    while t0 < tpc:
        stripes.append((t0, min(STRIPE_T, tpc - t0)))
        t0 += STRIPE_T

    order = np.lexsort((src, bank, tile, core))
    so, do_ = src[order], dst[order]
    co = core[order]
    to = tile[order]
    bo = bank[order]
    key = (co * tpc + to) * nbanks + bo
    ncell = N_CORES * tpc * nbanks
    cnts = np.bincount(key, minlength=ncell).reshape(N_CORES, tpc, nbanks)
    starts = np.zeros(ncell + 1, np.int64)
    np.cumsum(cnts.reshape(-1), out=starts[1:])

    runs = []
    total_slots = 0
    for si, (ts, nt) in enumerate(stripes):
        for b in range(nbanks):
            oc = 0
            for c in range(N_CORES):
                ov = int(np.maximum(cnts[c, ts:ts + nt, b] - q, 0).sum())
                oc = max(oc, ov)
            if oc > OVF:
                return None
            n_idx = nt * q + OVF
            n_reg = n_idx
            runs.append((si, b, nt, n_idx, n_reg))
            total_slots += n_idx
    n_chunks = total_slots // P
    w_tot = total_slots // 16

    idx_all = np.zeros((N_CORES, 16, w_tot), np.int16)
    dstl_all = np.full((N_CORES, P, n_chunks), -1.0, np.float32)

    wofs = 0
    cofs = 0
    run_offsets = []
    for (si, b, nt, n_idx, n_reg) in runs:
        ts = stripes[si][0]
        for c in range(N_CORES):
            idxs = np.full(n_idx, -1, np.int64)
            dls = np.full(n_idx, -1.0, np.float32)
            ovs_i, ovs_d = [], []
            for j, t in enumerate(range(ts, ts + nt)):
                k = (c * tpc + t) * nbanks + b
                s0, s1 = int(starts[k]), int(starts[k + 1])
                cell_src = so[s0:s1] - b * BANK
                cell_dst = do_[s0:s1] - (c * shard + t * P)
                take = min(s1 - s0, q)
                idxs[j * q:j * q + take] = cell_src[:take]
                dls[j * q:j * q + take] = cell_dst[:take]
                if s1 - s0 > take:
                    ovs_i.extend(cell_src[take:].tolist())
                    ovs_d.extend((cell_dst[take:] + j * P).tolist())
            no = len(ovs_i)
            base = nt * q
            if no:
                idxs[base:base + no] = ovs_i
                dls[base:base + no] = ovs_d
            # pads must still be gathered -- spread them over distinct rows
            # (same-row pads -> HBM bank hotspot, measurably slow)
            nrows = int(idxs.max()) + 1 if idxs.max() >= 0 else 1
            pad = idxs < 0
            if pad.any():
                idxs[pad] = (np.arange(int(pad.sum())) * 769) % nrows
            idx_all[c, :, wofs:wofs + n_idx // 16] = idxs.reshape(n_idx // 16, 16).T
            ch = n_idx // P
            dstl_all[c, :, cofs:cofs + ch] = dls.reshape(ch, P).T
        run_offsets.append((wofs, cofs, n_idx // P))
        wofs += n_idx // 16
        cofs += n_idx // P

    # pack idx (replicated to 128 partitions) + dstl (f32: per-partition
    # scalar operand of the one-hot tensor_scalar must be f32) into one
    # int16 tensor per run: [P, w_run + 2*ch_run] = ONE DMA load per run.
    comb_w = w_tot + 2 * n_chunks
    comb = np.zeros((N_CORES, P, comb_w), np.int16)
    comb_offsets = []
    co_ = 0
    for (wofs, cofs, ch), (si, b, nt, n_idx, n_reg) in zip(run_offsets, runs):
        w = n_idx // 16
        for c in range(N_CORES):
            comb[c, :, co_:co_ + w] = np.tile(idx_all[c, :, wofs:wofs + w],
                                              (8, 1))
            comb[c, :, co_ + w:co_ + w + 2 * ch] = (
                dstl_all[c, :, cofs:cofs + ch].view(np.int16))
        comb_offsets.append((co_, w, ch))
        co_ += w + 2 * ch
    assert co_ == comb_w
    return dict(runs=runs, comb_offsets=comb_offsets, stripes=stripes,
                comb=comb, nbanks=nbanks, q=q,
                n_chunks=n_chunks, w_tot=w_tot, comb_w=comb_w)


def _prep_type(ei, n_src_pad, tpc, q0):
    q = q0
    while True:
        r = _prep_edge_type(ei, n_src_pad, tpc, q)
        if r is not None:
            return r
        q += P


def _invcnt(ei, n_dst_pad, tpc):
    cnt = np.bincount(ei[1].astype(np.int64), minlength=n_dst_pad).astype(np.float32)
    inv = 1.0 / np.maximum(cnt, 1.0)
    return inv.reshape(N_CORES, tpc, P)


def build(inputs, reps=1):
    """Build the Bass program + per-core input maps. Returns (nc, in_maps, meta).

    reps > 1 unrolls the whole computation (proj + layers + collectives)
    that many times inside one program — for HW timing slope measurements."""
    import concourse.bass as bass
    import concourse.mybir as mybir
    import concourse.tile as tile
    import concourse.bacc as bacc

    import os as _os
    PROBE = set(p for p in _os.environ.get("BASS_PROBE", "").split(",") if p)
    import ml_dtypes
    bf16np = ml_dtypes.bfloat16

    f32 = mybir.dt.float32
    bf16 = mybir.dt.bfloat16
    fp16 = mybir.dt.float16
    i16 = mybir.dt.int16
    x_np = {"c": np.asarray(inputs["x_cheval"], np.float32),
            "j": np.asarray(inputs["x_jockey"], np.float32),
            "r": np.asarray(inputs["x_course"], np.float32)}
    NC = x_np["c"].shape[0]

    tpc = {k: _ceil(x_np[k].shape[0], P * N_CORES) for k in x_np}
    npad = {k: tpc[k] * P * N_CORES for k in tpc}
    # +1 input row of ones folds the input-proj bias in
    din = {k: x_np[k].shape[1] + 1 for k in x_np}

    xT = {}
    for k in x_np:
        xt = np.zeros((din[k], npad[k]), np.float32)
        xt[:-1, :x_np[k].shape[0]] = x_np[k].T
        xt[-1, :] = 1.0
        xT[k] = xt.astype(bf16np)

    w_in_np = {}
    for k, nm in (("c", "cheval"), ("j", "jockey"), ("r", "course")):
        w = np.asarray(inputs[f"w_in_{nm}"], np.float32)
        b = np.asarray(inputs[f"b_in_{nm}"], np.float32)
        w_in_np[k] = np.concatenate(
            [w, b.reshape(1, HID)], axis=0).astype(bf16np)

    w_cls = np.asarray(inputs["w_cls"], np.float32)
    b_cls = float(np.asarray(inputs["b_cls"]).reshape(-1)[0])
    eis = {k: np.asarray(inputs["ei_" + k]) for k in _ETYPES}
    NLAYERS = np.asarray(inputs["wl_part"]).shape[0]

    prep, iv = {}, {}
    for et, (dk, sk) in _ETYPES.items():
        lam = eis[et].shape[1] / (npad[dk] / P)
        nb = _ceil(npad[sk], BANK)
        q0 = max(P, _ceil(int(lam / nb) + 1, P) * P)
        prep[et] = _prep_type(eis[et], npad[sk], tpc[dk], q0)
        iv[et] = _invcnt(eis[et], npad[dk], tpc[dk])

    WL = {et: np.asarray(inputs["wl_" + et], np.float32) for et in _ETYPES}
    BL = {et: np.asarray(inputs["bl_" + et], np.float32) for et in _ETYPES}
    WR = {et: np.asarray(inputs["wr_" + et], np.float32) for et in _ETYPES}
    WRc = {dk: sum(WR[et] for et in _DST_ETYPES[dk]) for dk in _DST_ETYPES}
    Bc = {dk: sum(BL[et] for et in _DST_ETYPES[dk]) for dk in _DST_ETYPES}
    bias_nonzero = {dk: bool(np.any(Bc[dk])) for dk in _DST_ETYPES}

    nc = bacc.Bacc(None, num_swdge_queues=4)

    din_t = {k: nc.declare_dram_parameter(f"xT_{k}", [din[k], npad[k]], bf16, False)
             for k in tpc}
    xown_t = {k: nc.declare_dram_parameter(f"xo_{k}", [din[k], tpc[k] * P], bf16, False)
              for k in tpc}
    win_t = {k: nc.declare_dram_parameter(f"win_{k}", [din[k], HID], bf16, False)
             for k in tpc}
    comb_t = {et: nc.declare_dram_parameter(f"comb_{et}", [P, prep[et]["comb_w"]],
                                            i16, False) for et in _ETYPES}
    ivc_t = {et: nc.declare_dram_parameter(f"ivc_{et}", [P, tpc[_ETYPES[et][0]]],
                                           f32, False) for et in _ETYPES}
    wl_t = {et: nc.declare_dram_parameter(f"wl_{et}", [NLAYERS, HID, HID], bf16, False)
            for et in _ETYPES}
    wrc_t = {dk: nc.declare_dram_parameter(f"wrc_{dk}", [NLAYERS, HID, HID], bf16,
                                           False) for dk in _DST_ETYPES}
    bc_t = {dk: nc.declare_dram_parameter(f"bc_{dk}", [NLAYERS, 1, HID], bf16, False)
            for dk in _DST_ETYPES}
    iota128_t = nc.declare_dram_parameter("iota128", [P, P], fp16, False)
    iota512_t = nc.declare_dram_parameter("iota512", [P, 512], fp16, False)
    wclsr_t = nc.declare_dram_parameter("wclsr", [P, STRIPE_T * HID], bf16, False)
    out_t = nc.declare_dram_parameter("out", [tpc["c"] * P, 1], f32, True)


    with tile.TileContext(nc) as tc:
        with (
            tc.tile_pool(name="dram", bufs=1, space="DRAM") as dram,
            tc.tile_pool(name="wpool", bufs=1) as wpool,
            tc.tile_pool(name="gpool", bufs=3) as gpool,
            tc.tile_pool(name="ohpool", bufs=4) as ohpool,
            tc.tile_pool(name="pool", bufs=2) as pool,
            tc.tile_pool(name="psum", bufs=2, space="PSUM") as psum,
        ):
            # h tables use a "fat row" layout [rows, 128] bf16 with only
            # cols 0:64 real — gather descriptors must be 256B-aligned rows.
            FAT = 2 * HID
            h_full = {}
            hT_loc = {}
            shard_buf = {}
            ag_out = {}
            for k in tpc:
                h0 = nc.dram_tensor(f"h0{k}", [npad[k], FAT], bf16)
                ag_out[k] = [nc.dram_tensor(f"ag{k}{l}", [npad[k], FAT], bf16,
                                            addr_space="Shared")
                             for l in range(2)]
                h_full[k] = [h0, ag_out[k][0], ag_out[k][1]]
                hT_loc[k] = [nc.dram_tensor(f"hT{k}{l}", [HID, tpc[k] * P], bf16)
                             for l in range(3)]
                shard_buf[k] = [nc.dram_tensor(f"sh{k}{l}", [tpc[k] * P, FAT], bf16)
                                for l in range(2)]

            from concourse.masks import make_identity
            ident = wpool.tile([P, P], f32)
            make_identity(nc, ident[:])
            identb = wpool.tile([P, P], bf16)
            nc.vector.tensor_copy(identb[:], ident[:])
            iota128 = wpool.tile([P, P], fp16)
            nc.sync.dma_start(iota128[:], iota128_t[:])
            iota512 = wpool.tile([P, 512], fp16)
            nc.sync.dma_start(iota512[:], iota512_t[:])
            ones1 = wpool.tile([1, P], bf16)
            nc.gpsimd.memset(ones1[:], 1.0)
            wclsr2 = wpool.tile([P, STRIPE_T * HID], bf16)
            nc.sync.dma_start(wclsr2[:], wclsr_t[:])
            win_sb = {}
            for k in tpc:
                win_sb[k] = wpool.tile([din[k], HID], bf16, tag=f"win{k}", name=f"win{k}")
                nc.sync.dma_start(win_sb[k][:], win_t[k][:])
            wl_sb, wrc_sb, bc_sb = {}, {}, {}
            for et in _ETYPES:
                for l in range(NLAYERS):
                    wl_sb[(et, l)] = wpool.tile([HID, HID], bf16, tag=f"wl{et}{l}", name=f"wl{et}{l}")
                    nc.sync.dma_start(wl_sb[(et, l)][:], wl_t[et][l])
            for dk in _DST_ETYPES:
                for l in range(NLAYERS):
                    wrc_sb[(dk, l)] = wpool.tile([HID, HID], bf16, tag=f"wrc{dk}{l}", name=f"wrc{dk}{l}")
                    nc.sync.dma_start(wrc_sb[(dk, l)][:], wrc_t[dk][l])
                    bc_sb[(dk, l)] = wpool.tile([1, HID], bf16, tag=f"bc{dk}{l}", name=f"bc{dk}{l}")
                    nc.sync.dma_start(bc_sb[(dk, l)][:], bc_t[dk][l])

            # invcnt tables preloaded once (they are layer-invariant)
            ivall = {}
            for et in _ETYPES:
                dk = _ETYPES[et][0]
                ivall[et] = wpool.tile([P, tpc[dk]], f32, tag=f"iv{et}",
                                       name=f"iv{et}")
                nc.sync.dma_start(ivall[et][:], ivc_t[et][:])

            # hoist num_idxs register values (one RegisterMove per distinct
            # value instead of one per gather)
            nregs = {}
            for et in _ETYPES:
                for (si, b, nt_, n_idx, n_reg) in prep[et]["runs"]:
                    if n_reg not in nregs:
                        nregs[n_reg] = nc.gpsimd.to_reg(n_reg)

            # memset gather buffers once (stale-read safety under masks)
            g_chunks = {}
            for et in _ETYPES:
                mx = max(r[3] for r in prep[et]["runs"]) // P
                g_chunks[et] = mx
                for _ in range(3):
                    t_ = gpool.tile([P, mx * FAT], bf16, tag=f"g{et}")
                    nc.gpsimd.memset(t_[:], 0.0)

            qrot = [0]

            # ---- input projection (full, replicated) ----
            # r, j first: layer-0 cheval gathers only need the r/j tables,
            # so they can start while the big c projection still runs.
            GB = 8
            GBH = 4
            pj_marker = {}
            pj_htw = {}

            def input_proj():
                for k in ["r", "j", "c"]:
                    # own-shard h^T (root term source for layer 0)
                    for g0 in range(0, tpc[k], GBH):
                        gn = min(GBH, tpc[k] - g0)
                        xt = gpool.tile([din[k], GB * P], bf16, tag=f"xt{k}")
                        nc.sync.dma_start(xt[:, :gn * P],
                                          xown_t[k][:, g0 * P:(g0 + gn) * P])
                        ztp = psum.tile([HID, 512], f32, space="PSUM", tag="ztp")
                        for j in range(gn):
                            nc.tensor.matmul(
                                out=ztp[:, j * P:(j + 1) * P],
                                lhsT=win_sb[k][:],
                                rhs=xt[:, j * P:(j + 1) * P], start=(j == 0),
                                stop=(j == gn - 1), skip_group_check=True)
                        ztr = pool.tile([HID, STRIPE_T * P], bf16, tag="ztr")
                        nc.vector.tensor_scalar(
                            out=ztr[:, :gn * P], in0=ztp[:, :gn * P],
                            scalar1=0.0, scalar2=None, op0=mybir.AluOpType.max)
                        pj_htw[(k, g0 // GBH)] = nc.scalar.dma_start(
                            hT_loc[k][0][:, g0 * P:(g0 + gn) * P],
                            ztr[:, :gn * P])
                    # full replicated h0 table
                    h0w = []
                    ntile = npad[k] // P
                    for g0 in range(0, ntile, GB):
                        gn = min(GB, ntile - g0)
                        xt = gpool.tile([din[k], GB * P], bf16, tag=f"xt{k}")
                        nc.sync.dma_start(xt[:, :gn * P],
                                          din_t[k][:, g0 * P:(g0 + gn) * P])
                        zp = psum.tile([P, 512], f32, space="PSUM", tag="zmr")
                        for j in range(gn):
                            nc.tensor.matmul(
                                out=zp[:, j * HID:(j + 1) * HID],
                                lhsT=xt[:, j * P:(j + 1) * P],
                                rhs=win_sb[k][:], start=(j == 0), stop=(j == gn - 1),
                                skip_group_check=True)
                        zr = pool.tile([P, GB * HID], bf16, tag="zr")
                        nc.vector.tensor_scalar(
                            out=zr[:, :gn * HID], in0=zp[:, :gn * HID],
                            scalar1=0.0, scalar2=None, op0=mybir.AluOpType.max)
                        h0w.append(nc.scalar.dma_start(
                            h_full[k][0][:][g0 * P:(g0 + gn) * P, :HID].rearrange(
                                "(t p) f -> p t f", p=P),
                            zr[:, :gn * HID].rearrange("p (t f) -> p t f", f=HID)))
                    mk = nc.sync.nop()
                    for wi in h0w:
                        add_dep_helper(mk.ins, wi.ins, sync=True,
                                       reason=f"proj-{k}-table-done")
                    pj_marker[k] = mk

            # targeted cross-layer dependencies (instead of full barriers):
            #   sbw: shard_buf writes feeding each AllGather
            #   htw: hT_loc stripe writes read by next layer's root term
            #   ag_comm: the AllGather instruction for (node type, layer)
            from concourse.tile import add_dep_helper
            sbw = {}
            htw = {}
            ag_comm = {}

            def do_layer(l):
                last = (l == NLAYERS - 1)
                # order dsts so each AllGather launches as early as possible
                # and overlaps the remaining dsts' compute:
                #   l=0: c first (its AG gates l=1 r/j); l=1: r,j first
                #   (their AGs gate l=2), c (no AG) last.
                if last:
                    dks = ["c"]
                elif l == 0:
                    dks = ["c", "r", "j"]
                else:
                    dks = ["r", "j", "c"]
                for dk in dks:
                    sbw.pop((dk, l), None)
                    ets = _DST_ETYPES[dk]
                    stripes = prep[ets[0]]["stripes"]
                    # skip shard/AG for c at layer NLAYERS-2: layer NLAYERS-1
                    # never gathers from the c table
                    need_ag = (not last) and not (dk == "c" and l == NLAYERS - 2)
                    for si, (ts, nt) in enumerate(stripes):
                        aggs = {}
                        for eti, et in enumerate(ets):
                            pr = prep[et]
                            agg = psum.tile([HID, 512], f32, space="PSUM",
                                            tag=f"agg{eti}")
                            aggs[et] = agg
                            rlist = [(i, r) for i, r in enumerate(pr["runs"])
                                     if r[0] == si]
                            n_r = len(rlist)
                            q = pr["q"]
                            cpt = q // P
                            for ri, (gi, (_, b, nt_, n_idx, n_reg)) in enumerate(rlist):
                                cofs_, w, ch = pr["comb_offsets"][gi]
                                it = gpool.tile([P, w + 2 * ch], mybir.dt.int16,
                                                tag=f"it{et}")
                                nc.sync.dma_start(
                                    it[:], comb_t[et][:, cofs_:cofs_ + w + 2 * ch])
                                dt_ = it[:, w:w + 2 * ch].bitcast(f32)
                                gt = gpool.tile([P, g_chunks[et] * FAT], bf16,
                                                tag=f"g{et}")
                                sk = _ETYPES[et][1]
                                tab = h_full[sk][l]
                                b_hi = min((b + 1) * BANK, npad[sk])
                                if "nogather" not in PROBE:
                                    gin = nc.gpsimd.dma_gather(
                                        out_ap=gt[:, :ch * FAT].rearrange(
                                            "p (c f) -> p c f", f=FAT),
                                        in_ap=tab[:][b * BANK:b_hi, :],
                                        idxs_ap=it[:, :w],
                                        num_idxs=n_idx, num_idxs_reg=nregs[n_reg],
                                        elem_size=FAT, single_packet=False,
                                        queue_num=qrot[0] % 4)
                                    if l >= 1 and (sk, l - 1) in ag_comm:
                                        add_dep_helper(
                                            gin.ins, ag_comm[(sk, l - 1)].ins,
                                            sync=True, reason="gather-after-ag")
                                    elif l == 0 and sk in pj_marker:
                                        add_dep_helper(
                                            gin.ins, pj_marker[sk].ins,
                                            sync=True, reason="gather-after-proj")
                                qrot[0] += 1
                                # one-hot per chunk: tensor_scalar with the
                                # dstl column as per-partition f32 scalar --
                                # iota/oh fp16 qualify for DVE fast modes
                                for c in range(ch):
                                    is_ovf = (c >= nt_ * cpt)
                                    wdt = nt_ * P if is_ovf else P
                                    io = iota512 if is_ovf else iota128
                                    oh = ohpool.tile([P, 512], fp16, tag="oh")
                                    nc.vector.tensor_scalar(
                                        out=oh[:, :wdt], in0=io[:, :wdt],
                                        scalar1=dt_[:, c:c + 1], scalar2=None,
                                        op0=mybir.AluOpType.is_equal)
                                    if is_ovf:
                                        lastmm = (ri == n_r - 1)
                                        nc.tensor.matmul(
                                            out=agg[:, :wdt],
                                            lhsT=gt[:, c * FAT:c * FAT + HID],
                                            rhs=oh[:, :wdt],
                                            start=False, stop=lastmm,
                                            skip_group_check=True)
                                    else:
                                        tt = c // cpt
                                        o0, o1 = tt * P, (tt + 1) * P
                                        first = (ri == 0 and c == 0)
                                        nc.tensor.matmul(
                                            out=agg[:, o0:o1],
                                            lhsT=gt[:, c * FAT:c * FAT + HID],
                                            rhs=oh[:, :wdt],
                                            start=first, stop=False,
                                            skip_group_check=True)
                        aggsb = {}
                        for et in ets:
                            a = pool.tile([HID, 512], bf16, tag=f"aggsb{et}",
                                          name=f"aggsb{et}")
                            nc.vector.tensor_copy(a[:, :nt * P],
                                                  aggs[et][:, :nt * P])
                            aggsb[et] = a
                        # root-term h^T for the whole stripe in one DMA
                        hTs = pool.tile([HID, STRIPE_T * P], bf16, tag="hTt")
                        hli = nc.scalar.dma_start(
                            hTs[:, :nt * P],
                            hT_loc[dk][l][:][:, ts * P:(ts + nt) * P])
                        if l >= 1 and (dk, l, si) in htw:
                            add_dep_helper(hli.ins, htw[(dk, l, si)].ins,
                                           sync=True, reason="hT-after-write")
                        elif l == 0:
                            for g in range(ts // GBH,
                                           (ts + nt - 1) // GBH + 1):
                                if (dk, g) in pj_htw:
                                    add_dep_helper(
                                        hli.ins, pj_htw[(dk, g)].ins,
                                        sync=True, reason="hT-after-proj")
                        zsb = pool.tile([P, STRIPE_T * HID], bf16, tag="zsb")
                        if not last:
                            ztr2 = pool.tile([HID, STRIPE_T * P], bf16, tag="ztr2")
                        for j in range(nt):
                            t = ts + j
                            zmr = psum.tile([P, 256], f32, space="PSUM",
                                            tag="zmr")
                            for ei_, et in enumerate(ets):
                                nc.tensor.matmul(
                                    out=zmr[:, ei_ * HID:(ei_ + 1) * HID],
                                    lhsT=aggsb[et][:, j * P:(j + 1) * P],
                                    rhs=wl_sb[(et, l)][:],
                                    start=(ei_ == 0), stop=False,
                                    skip_group_check=True)
                            ro = 2 * HID
                            nc.tensor.matmul(out=zmr[:, ro:ro + HID],
                                             lhsT=hTs[:, j * P:(j + 1) * P],
                                             rhs=wrc_sb[(dk, l)][:],
                                             start=False,
                                             stop=not bias_nonzero[dk],
                                             skip_group_check=True)
                            if bias_nonzero[dk]:
                                nc.tensor.matmul(out=zmr[:, ro:ro + HID],
                                                 lhsT=ones1[:],
                                                 rhs=bc_sb[(dk, l)][:],
                                                 start=False, stop=True,
                                                 skip_group_check=True)
                            # z = sum_et ivc_et * zm_et + zroot, then relu
                            zrt = pool.tile([P, HID], f32, tag="zrt")
                            nc.vector.tensor_copy(zrt[:], zmr[:, ro:ro + HID])
                            tmp = pool.tile([P, HID], f32, tag="ztmp")
                            nc.vector.scalar_tensor_tensor(
                                out=tmp[:],
                                in0=zmr[:, 0:HID],
                                scalar=ivall[ets[0]][:, t:t + 1],
                                in1=zrt[:],
                                op0=mybir.AluOpType.mult,
                                op1=mybir.AluOpType.add)
                            if len(ets) > 1:
                                nc.vector.scalar_tensor_tensor(
                                    out=tmp[:],
                                    in0=zmr[:, HID:2 * HID],
                                    scalar=ivall[ets[1]][:, t:t + 1],
                                    in1=tmp[:],
                                    op0=mybir.AluOpType.mult,
                                    op1=mybir.AluOpType.add)
                            nc.vector.tensor_scalar(
                                out=zsb[:, j * HID:(j + 1) * HID], in0=tmp[:],
                                scalar1=0.0, scalar2=None,
                                op0=mybir.AluOpType.max)
                            if not last:
                                ztp = psum.tile([HID, 512], bf16, space="PSUM",
                                                tag="ztp")
                                nc.tensor.transpose(
                                    out=ztp[:, :P],
                                    in_=zsb[:, j * HID:(j + 1) * HID],
                                    identity=identb[:])
                                nc.vector.tensor_copy(
                                    ztr2[:, j * P:(j + 1) * P], ztp[:, :P])
                        if not last:
                            htw[(dk, l + 1, si)] = nc.scalar.dma_start(
                                hT_loc[dk][l + 1][:][:, ts * P:(ts + nt) * P],
                                ztr2[:, :nt * P])
                            if need_ag:
                                wi = nc.scalar.dma_start(
                                    shard_buf[dk][l][:][ts * P:(ts + nt) * P, :HID]
                                    .rearrange("(t p) f -> p t f", p=P),
                                    zsb[:, :nt * HID].rearrange(
                                        "p (t f) -> p t f", f=HID))
                                sbw.setdefault((dk, l), []).append(wi)
                        else:
                            tmp2 = pool.tile([P, STRIPE_T * HID], f32, tag="ctmp")
                            nc.vector.tensor_tensor(
                                out=tmp2[:, :nt * HID], in0=zsb[:, :nt * HID],
                                in1=wclsr2[:, :nt * HID],
                                op=mybir.AluOpType.mult)
                            ot = pool.tile([P, STRIPE_T], f32, tag="otile")
                            nc.vector.tensor_reduce(
                                out=ot[:, :nt],
                                in_=tmp2[:, :nt * HID].rearrange(
                                    "p (t f) -> p t f", f=HID),
                                axis=mybir.AxisListType.X,
                                op=mybir.AluOpType.add)
                            if b_cls != 0.0:
                                nc.vector.tensor_scalar(
                                    out=ot[:, :nt], in0=ot[:, :nt],
                                    scalar1=b_cls, scalar2=None,
                                    op0=mybir.AluOpType.add)
                            oap = out_t[:].rearrange("(t p) o -> p t o", p=P)
                            nc.scalar.dma_start(oap[:, ts:ts + nt, 0], ot[:, :nt])
                    if need_ag and "noag" not in PROBE:
                        cc = nc.gpsimd.collective_compute(
                            "AllGather", mybir.AluOpType.bypass,
                            ins=[shard_buf[dk][l][:]],
                            outs=[ag_out[dk][l][:]],
                            replica_groups=[list(range(N_CORES))])
                        for wi in sbw.get((dk, l), []):
                            add_dep_helper(cc.ins, wi.ins, sync=True,
                                           reason="ag-after-shard-writes")
                        ag_comm[(dk, l)] = cc

            for _rep in range(reps):
                if "noproj" not in PROBE:
                    input_proj()
                for l in range(NLAYERS):
                    do_layer(l)

    nc.finalize()

    iota128_v = np.broadcast_to(np.arange(P, dtype=np.float16), (P, P)).copy()
    iota512_v = np.broadcast_to(np.arange(512, dtype=np.float16), (P, 512)).copy()
    wclsr_v = np.tile(w_cls.reshape(1, HID), (P, STRIPE_T)).astype(bf16np)

    in_maps = []
    for c in range(N_CORES):
        m = {}
        for k in tpc:
            sh = tpc[k] * P
            m[f"xT_{k}"] = xT[k]
            m[f"xo_{k}"] = np.ascontiguousarray(xT[k][:, c * sh:(c + 1) * sh])
            m[f"win_{k}"] = w_in_np[k]
        for et in _ETYPES:
            m[f"comb_{et}"] = prep[et]["comb"][c]
            m[f"ivc_{et}"] = np.ascontiguousarray(iv[et][c].T)
            m[f"wl_{et}"] = WL[et].astype(bf16np)
        for dk in _DST_ETYPES:
            m[f"wrc_{dk}"] = np.asarray(WRc[dk], np.float32).astype(bf16np)
            m[f"bc_{dk}"] = np.asarray(Bc[dk], np.float32).reshape(
                NLAYERS, 1, HID).astype(bf16np)
        m["iota128"] = iota128_v
        m["iota512"] = iota512_v
        m["wclsr"] = wclsr_v
        in_maps.append(m)

    return nc, in_maps, dict(tpc=tpc, NC=NC)


def kernel(**inputs):
    import jax
    from jax.sharding import Mesh, PartitionSpec, NamedSharding
    from jax.experimental.shard_map import shard_map
    from concourse.bass2jax import (_bass_exec_p, partition_id_tensor,
                                    install_neuronx_cc_hook)
    import concourse.mybir as mybir

    nc, in_maps, meta = build(inputs)
    tpc, NC = meta["tpc"], meta["NC"]

    install_neuronx_cc_hook()
    partition_name = nc.partition_id_tensor.name if nc.partition_id_tensor else None
    in_names, out_names, out_avals, zero_outs = [], [], [], []
    for alloc in nc.m.functions[0].allocations:
        if not isinstance(alloc, mybir.MemoryLocationSet):
            continue
        name = alloc.memorylocations[0].name
        if alloc.kind == "ExternalInput":
            if name != partition_name:
                in_names.append(name)
        elif alloc.kind == "ExternalOutput":
            out_names.append(name)
            shape = tuple(alloc.tensor_shape)
            dtype = mybir.dt.np(alloc.dtype)
            out_avals.append(jax.core.ShapedArray(shape, dtype))
            zero_outs.append(np.zeros(shape, dtype))
    n_params = len(in_names)
    all_in = list(in_names) + list(out_names)
    if partition_name is not None:
        all_in.append(partition_name)

    def _body(*args):
        operands = list(args)
        if partition_name is not None:
            operands.append(partition_id_tensor())
        outs = _bass_exec_p.bind(
            *operands, out_avals=tuple(out_avals), in_names=tuple(all_in),
            out_names=tuple(out_names), lowering_input_output_aliases=(),
            sim_require_finite=False, sim_require_nnan=False, nc=nc)
        return tuple(outs)

    devices = jax.devices()[:N_CORES]
    mesh = Mesh(np.asarray(devices), ("core",))
    specs = (PartitionSpec("core"),)
    sharded = jax.jit(
        shard_map(_body, mesh=mesh, in_specs=specs * (n_params + len(out_names)),
                  out_specs=specs * len(out_names), check_rep=False),
        keep_unused=True)
    per_core = [[np.asarray(m[n]) for n in in_names] for m in in_maps]
    concat_in = [np.concatenate([per_core[c][i] for c in range(N_CORES)], axis=0)
                 for i in range(n_params)]
    concat_zero = [np.zeros((N_CORES * z.shape[0], *z.shape[1:]), z.dtype)
                   for z in zero_outs]
    shd = NamedSharding(mesh, PartitionSpec("core"))
    dev_in = [
        jax.make_array_from_callback(a.shape, shd, lambda idx, a=a: a[idx])
        for a in concat_in + concat_zero
    ]
    outs = sharded(*dev_in)
    jax.block_until_ready(outs)
    import os as _os
    if _os.environ.get("BASS_KERNEL_TIME"):
        import time as _time
        times = []
        for _ in range(int(_os.environ.get("BASS_KERNEL_REPS", "8"))):
            t0 = _time.perf_counter()
            outs2 = sharded(*dev_in)
            jax.block_until_ready(outs2)
            times.append(_time.perf_counter() - t0)
        print(f"HW exec time: {min(times) * 1e9:.0f} ns")
        print(f"exec times (s): {[f'{t:.4f}' for t in times]}")
    oi = out_names.index("out")
    full = np.asarray(outs[oi]).reshape(N_CORES * tpc["c"] * P, 1)
    return full[:NC, :].astype(np.float32)


# revision 8
# speedup vs baseline: 2.2964x; 1.1774x over previous
"""Trainium2 Bass kernel for 3-layer heterogeneous GraphSAGE (EntityGraphNN).

8 NeuronCores, SPMD single program:
  - Destination-node sharding: each core owns 1/8 of each node type's
    128-row tiles. Edges routed to the core owning their dst.
  - h[src] row gathers (256B) via gpsimd dma_gather over 4 SWDGE queues.
    Sources banked by 32768 rows (int16 index limit). One gather per
    (stripe-of-4-dst-tiles, src-bank): fixed quota Q slots per (tile, bank)
    cell + one 128-slot overflow chunk (trailing -1 idxs are skipped).
  - Scatter-add via one-hot matmul: oh[e,d] = (dstl[e]==d) on DVE, PSUM
    agg_T[64, 512] accumulates G^T @ oh per stripe.
  - mean = agg * invcnt (host-precomputed), z = mean@Wl + h_dst@Wr + b and
    its transpose z_T by the swapped matmul pair; ReLU; h stored row-major
    (gather table) and transposed (next layer's root term).
  - Input projections replicated on every core (no layer-0 AllGather).
  - AllGather of h after layer 0 (all types) and layer 1 (r, j only --
    layer 2 computes only cheval logits and never gathers from c).
  - idx+dstl packed in one int16 load per run; invcnt preloaded once;
    per-stripe batched DMAs for shard/hT writes and hT reads.
"""
import numpy as np

HID = 64
P = 128
BANK = 32768
N_CORES = 8
STRIPE_T = 4
OVF = 128

_ETYPES = {
    "rev_part": ("c", "r"),
    "monte": ("c", "j"),
    "part": ("r", "c"),
    "rev_monte": ("j", "c"),
}
_DST_ETYPES = {"c": ["rev_part", "monte"], "r": ["part"], "j": ["rev_monte"]}


def _ceil(a, b):
    return (a + b - 1) // b


def _prep_edge_type(ei, n_src_pad, tpc, q):
    src = ei[0].astype(np.int64)
    dst = ei[1].astype(np.int64)
    nbanks = _ceil(n_src_pad, BANK)
    shard = tpc * P
    core = dst // shard
    tile = (dst % shard) // P
    bank = src // BANK

    stripes = []
    t0 = 0
    while t0 < tpc:
        stripes.append((t0, min(STRIPE_T, tpc - t0)))
        t0 += STRIPE_T

    order = np.lexsort((src, bank, tile, core))
    so, do_ = src[order], dst[order]
    co = core[order]
    to = tile[order]
    bo = bank[order]
    key = (co * tpc + to) * nbanks + bo
    ncell = N_CORES * tpc * nbanks
    cnts = np.bincount(key, minlength=ncell).reshape(N_CORES, tpc, nbanks)
    starts = np.zeros(ncell + 1, np.int64)
    np.cumsum(cnts.reshape(-1), out=starts[1:])

    runs = []
    total_slots = 0
    for si, (ts, nt) in enumerate(stripes):
        for b in range(nbanks):
            oc = 0
            for c in range(N_CORES):
                ov = int(np.maximum(cnts[c, ts:ts + nt, b] - q, 0).sum())
                oc = max(oc, ov)
            ovf_cap = _ceil(oc, P) * P
            if ovf_cap > 1024:
                return None
            n_idx = nt * q + ovf_cap
            n_reg = n_idx
            runs.append((si, b, nt, n_idx, n_reg))
            total_slots += n_idx
    n_chunks = total_slots // P
    w_tot = total_slots // 16

    idx_all = np.zeros((N_CORES, 16, w_tot), np.int16)
    dstl_all = np.full((N_CORES, P, n_chunks), -1.0, np.float32)

    wofs = 0
    cofs = 0
    run_offsets = []
    for (si, b, nt, n_idx, n_reg) in runs:
        ts = stripes[si][0]
        for c in range(N_CORES):
            idxs = np.full(n_idx, -1, np.int64)
            dls = np.full(n_idx, -1.0, np.float32)
            ovs_i, ovs_d = [], []
            for j, t in enumerate(range(ts, ts + nt)):
                k = (c * tpc + t) * nbanks + b
                s0, s1 = int(starts[k]), int(starts[k + 1])
                cell_src = so[s0:s1] - b * BANK
                cell_dst = do_[s0:s1] - (c * shard + t * P)
                take = min(s1 - s0, q)
                idxs[j * q:j * q + take] = cell_src[:take]
                dls[j * q:j * q + take] = cell_dst[:take]
                if s1 - s0 > take:
                    ovs_i.extend(cell_src[take:].tolist())
                    ovs_d.extend((cell_dst[take:] + j * P).tolist())
            no = len(ovs_i)
            base = nt * q
            if no:
                idxs[base:base + no] = ovs_i
                dls[base:base + no] = ovs_d
            # pads must still be gathered -- spread them over distinct rows
            # (same-row pads -> HBM bank hotspot, measurably slow)
            nrows = int(idxs.max()) + 1 if idxs.max() >= 0 else 1
            pad = idxs < 0
            if pad.any():
                idxs[pad] = (np.arange(int(pad.sum())) * 769) % nrows
            idx_all[c, :, wofs:wofs + n_idx // 16] = idxs.reshape(n_idx // 16, 16).T
            ch = n_idx // P
            dstl_all[c, :, cofs:cofs + ch] = dls.reshape(ch, P).T
        run_offsets.append((wofs, cofs, n_idx // P))
        wofs += n_idx // 16
        cofs += n_idx // P

    # pack idx (replicated to 128 partitions) + dstl (f32: per-partition
    # scalar operand of the one-hot tensor_scalar must be f32) into one
    # int16 tensor per run: [P, w_run + 2*ch_run] = ONE DMA load per run.
    comb_w = w_tot + 2 * n_chunks
    comb = np.zeros((N_CORES, P, comb_w), np.int16)
    comb_offsets = []
    co_ = 0
    for (wofs, cofs, ch), (si, b, nt, n_idx, n_reg) in zip(run_offsets, runs):
        w = n_idx // 16
        for c in range(N_CORES):
            comb[c, :, co_:co_ + w] = np.tile(idx_all[c, :, wofs:wofs + w],
                                              (8, 1))
            comb[c, :, co_ + w:co_ + w + 2 * ch] = (
                dstl_all[c, :, cofs:cofs + ch].view(np.int16))
        comb_offsets.append((co_, w, ch))
        co_ += w + 2 * ch
    assert co_ == comb_w
    return dict(runs=runs, comb_offsets=comb_offsets, stripes=stripes,
                comb=comb, nbanks=nbanks, q=q,
                n_chunks=n_chunks, w_tot=w_tot, comb_w=comb_w)


def _prep_type(ei, n_src_pad, tpc, q0):
    q = q0
    while True:
        r = _prep_edge_type(ei, n_src_pad, tpc, q)
        if r is not None:
            return r
        q += P


def _invcnt(ei, n_dst_pad, tpc):
    cnt = np.bincount(ei[1].astype(np.int64), minlength=n_dst_pad).astype(np.float32)
    inv = 1.0 / np.maximum(cnt, 1.0)
    return inv.reshape(N_CORES, tpc, P)


def build(inputs, reps=1):
    """Build the Bass program + per-core input maps. Returns (nc, in_maps, meta).

    reps > 1 unrolls the whole computation (proj + layers + collectives)
    that many times inside one program — for HW timing slope measurements."""
    import concourse.bass as bass
    import concourse.mybir as mybir
    import concourse.tile as tile
    import concourse.bacc as bacc

    import os as _os
    PROBE = set(p for p in _os.environ.get("BASS_PROBE", "").split(",") if p)
    import ml_dtypes
    bf16np = ml_dtypes.bfloat16

    f32 = mybir.dt.float32
    bf16 = mybir.dt.bfloat16
    fp16 = mybir.dt.float16
    i16 = mybir.dt.int16
    x_np = {"c": np.asarray(inputs["x_cheval"], np.float32),
            "j": np.asarray(inputs["x_jockey"], np.float32),
            "r": np.asarray(inputs["x_course"], np.float32)}
    NC = x_np["c"].shape[0]

    tpc = {k: _ceil(x_np[k].shape[0], P * N_CORES) for k in x_np}
    npad = {k: tpc[k] * P * N_CORES for k in tpc}
    # +1 input row of ones folds the input-proj bias in
    din = {k: x_np[k].shape[1] + 1 for k in x_np}

    xT = {}
    for k in x_np:
        xt = np.zeros((din[k], npad[k]), np.float32)
        xt[:-1, :x_np[k].shape[0]] = x_np[k].T
        xt[-1, :] = 1.0
        xT[k] = xt.astype(bf16np)

    w_in_np = {}
    for k, nm in (("c", "cheval"), ("j", "jockey"), ("r", "course")):
        w = np.asarray(inputs[f"w_in_{nm}"], np.float32)
        b = np.asarray(inputs[f"b_in_{nm}"], np.float32)
        w_in_np[k] = np.concatenate(
            [w, b.reshape(1, HID)], axis=0).astype(bf16np)

    w_cls = np.asarray(inputs["w_cls"], np.float32)
    b_cls = float(np.asarray(inputs["b_cls"]).reshape(-1)[0])
    eis = {k: np.asarray(inputs["ei_" + k]) for k in _ETYPES}
    NLAYERS = np.asarray(inputs["wl_part"]).shape[0]

    prep, iv = {}, {}
    for et, (dk, sk) in _ETYPES.items():
        lam = eis[et].shape[1] / (npad[dk] / P)
        nb = _ceil(npad[sk], BANK)
        q0 = max(P, (int(lam / nb) // P) * P)
        prep[et] = _prep_type(eis[et], npad[sk], tpc[dk], q0)
        iv[et] = _invcnt(eis[et], npad[dk], tpc[dk])

    WL = {et: np.asarray(inputs["wl_" + et], np.float32) for et in _ETYPES}
    BL = {et: np.asarray(inputs["bl_" + et], np.float32) for et in _ETYPES}
    WR = {et: np.asarray(inputs["wr_" + et], np.float32) for et in _ETYPES}
    WRc = {dk: sum(WR[et] for et in _DST_ETYPES[dk]) for dk in _DST_ETYPES}
    Bc = {dk: sum(BL[et] for et in _DST_ETYPES[dk]) for dk in _DST_ETYPES}
    bias_nonzero = {dk: bool(np.any(Bc[dk])) for dk in _DST_ETYPES}

    nc = bacc.Bacc(None, num_swdge_queues=4)

    din_t = {k: nc.declare_dram_parameter(f"xT_{k}", [din[k], npad[k]], bf16, False)
             for k in tpc}
    xown_t = {k: nc.declare_dram_parameter(f"xo_{k}", [din[k], tpc[k] * P], bf16, False)
              for k in tpc}
    win_t = {k: nc.declare_dram_parameter(f"win_{k}", [din[k], HID], bf16, False)
             for k in tpc}
    comb_t = {et: nc.declare_dram_parameter(f"comb_{et}", [P, prep[et]["comb_w"]],
                                            i16, False) for et in _ETYPES}
    ivc_t = {et: nc.declare_dram_parameter(f"ivc_{et}", [P, tpc[_ETYPES[et][0]]],
                                           f32, False) for et in _ETYPES}
    wl_t = {et: nc.declare_dram_parameter(f"wl_{et}", [NLAYERS, HID, HID], bf16, False)
            for et in _ETYPES}
    wrc_t = {dk: nc.declare_dram_parameter(f"wrc_{dk}", [NLAYERS, HID, HID], bf16,
                                           False) for dk in _DST_ETYPES}
    bc_t = {dk: nc.declare_dram_parameter(f"bc_{dk}", [NLAYERS, 1, HID], bf16, False)
            for dk in _DST_ETYPES}
    iota128_t = nc.declare_dram_parameter("iota128", [P, P], fp16, False)
    iota512_t = nc.declare_dram_parameter("iota512", [P, 512], fp16, False)
    wclsr_t = nc.declare_dram_parameter("wclsr", [P, STRIPE_T * HID], bf16, False)
    out_t = nc.declare_dram_parameter("out", [tpc["c"] * P, 1], f32, True)


    with tile.TileContext(nc) as tc:
        with (
            tc.tile_pool(name="dram", bufs=1, space="DRAM") as dram,
            tc.tile_pool(name="wpool", bufs=1) as wpool,
            tc.tile_pool(name="gpool", bufs=3) as gpool,
            tc.tile_pool(name="ohpool", bufs=4) as ohpool,
            tc.tile_pool(name="pool", bufs=2) as pool,
            tc.tile_pool(name="psum", bufs=2, space="PSUM") as psum,
        ):
            # h tables use a "fat row" layout [rows, 128] bf16 with only
            # cols 0:64 real — gather descriptors must be 256B-aligned rows.
            FAT = 2 * HID
            h_full = {}
            hT_loc = {}
            shard_buf = {}
            ag_out = {}
            for k in tpc:
                h0 = nc.dram_tensor(f"h0{k}", [npad[k], FAT], bf16)
                ag_out[k] = [nc.dram_tensor(f"ag{k}{l}", [npad[k], FAT], bf16,
                                            addr_space="Shared")
                             for l in range(2)]
                h_full[k] = [h0, ag_out[k][0], ag_out[k][1]]
                hT_loc[k] = [nc.dram_tensor(f"hT{k}{l}", [HID, tpc[k] * P], bf16)
                             for l in range(3)]
                shard_buf[k] = [nc.dram_tensor(f"sh{k}{l}", [tpc[k] * P, FAT], bf16)
                                for l in range(2)]

            from concourse.masks import make_identity
            ident = wpool.tile([P, P], f32)
            make_identity(nc, ident[:])
            identb = wpool.tile([P, P], bf16)
            nc.vector.tensor_copy(identb[:], ident[:])
            iota128 = wpool.tile([P, P], fp16)
            nc.sync.dma_start(iota128[:], iota128_t[:])
            iota512 = wpool.tile([P, 512], fp16)
            nc.sync.dma_start(iota512[:], iota512_t[:])
            ones1 = wpool.tile([1, P], bf16)
            nc.gpsimd.memset(ones1[:], 1.0)
            wclsr2 = wpool.tile([P, STRIPE_T * HID], bf16)
            nc.sync.dma_start(wclsr2[:], wclsr_t[:])
            win_sb = {}
            for k in tpc:
                win_sb[k] = wpool.tile([din[k], HID], bf16, tag=f"win{k}", name=f"win{k}")
                nc.sync.dma_start(win_sb[k][:], win_t[k][:])
            wl_sb, wrc_sb, bc_sb = {}, {}, {}
            for et in _ETYPES:
                for l in range(NLAYERS):
                    wl_sb[(et, l)] = wpool.tile([HID, HID], bf16, tag=f"wl{et}{l}", name=f"wl{et}{l}")
                    nc.sync.dma_start(wl_sb[(et, l)][:], wl_t[et][l])
            for dk in _DST_ETYPES:
                for l in range(NLAYERS):
                    wrc_sb[(dk, l)] = wpool.tile([HID, HID], bf16, tag=f"wrc{dk}{l}", name=f"wrc{dk}{l}")
                    nc.sync.dma_start(wrc_sb[(dk, l)][:], wrc_t[dk][l])
                    bc_sb[(dk, l)] = wpool.tile([1, HID], bf16, tag=f"bc{dk}{l}", name=f"bc{dk}{l}")
                    nc.sync.dma_start(bc_sb[(dk, l)][:], bc_t[dk][l])

            # invcnt tables preloaded once (they are layer-invariant)
            ivall = {}
            for et in _ETYPES:
                dk = _ETYPES[et][0]
                ivall[et] = wpool.tile([P, tpc[dk]], f32, tag=f"iv{et}",
                                       name=f"iv{et}")
                nc.sync.dma_start(ivall[et][:], ivc_t[et][:])

            # hoist num_idxs register values (one RegisterMove per distinct
            # value instead of one per gather)
            nregs = {}
            for et in _ETYPES:
                for (si, b, nt_, n_idx, n_reg) in prep[et]["runs"]:
                    if n_reg not in nregs:
                        nregs[n_reg] = nc.gpsimd.to_reg(n_reg)

            # memset gather buffers once (stale-read safety under masks)
            g_chunks = {}
            for et in _ETYPES:
                mx = max(r[3] for r in prep[et]["runs"]) // P
                g_chunks[et] = mx
                for _ in range(3):
                    t_ = gpool.tile([P, mx * FAT], bf16, tag=f"g{et}")
                    nc.gpsimd.memset(t_[:], 0.0)

            qrot = [0]

            # ---- input projection (full, replicated) ----
            # r, j first: layer-0 cheval gathers only need the r/j tables,
            # so they can start while the big c projection still runs.
            GB = 8
            GBH = 4
            pj_marker = {}
            pj_htw = {}

            def input_proj():
                for k in ["r", "j", "c"]:
                    # own-shard h^T (root term source for layer 0)
                    for g0 in range(0, tpc[k], GBH):
                        gn = min(GBH, tpc[k] - g0)
                        xt = gpool.tile([din[k], GB * P], bf16, tag=f"xt{k}")
                        nc.sync.dma_start(xt[:, :gn * P],
                                          xown_t[k][:, g0 * P:(g0 + gn) * P])
                        ztp = psum.tile([HID, 512], f32, space="PSUM", tag="ztp")
                        for j in range(gn):
                            nc.tensor.matmul(
                                out=ztp[:, j * P:(j + 1) * P],
                                lhsT=win_sb[k][:],
                                rhs=xt[:, j * P:(j + 1) * P], start=(j == 0),
                                stop=(j == gn - 1), skip_group_check=True)
                        ztr = pool.tile([HID, STRIPE_T * P], bf16, tag="ztr")
                        nc.vector.tensor_scalar(
                            out=ztr[:, :gn * P], in0=ztp[:, :gn * P],
                            scalar1=0.0, scalar2=None, op0=mybir.AluOpType.max)
                        pj_htw[(k, g0 // GBH)] = nc.scalar.dma_start(
                            hT_loc[k][0][:, g0 * P:(g0 + gn) * P],
                            ztr[:, :gn * P])
                    # full replicated h0 table
                    h0w = []
                    ntile = npad[k] // P
                    for g0 in range(0, ntile, GB):
                        gn = min(GB, ntile - g0)
                        xt = gpool.tile([din[k], GB * P], bf16, tag=f"xt{k}")
                        nc.sync.dma_start(xt[:, :gn * P],
                                          din_t[k][:, g0 * P:(g0 + gn) * P])
                        zp = psum.tile([P, 512], f32, space="PSUM", tag="zmr")
                        for j in range(gn):
                            nc.tensor.matmul(
                                out=zp[:, j * HID:(j + 1) * HID],
                                lhsT=xt[:, j * P:(j + 1) * P],
                                rhs=win_sb[k][:], start=(j == 0), stop=(j == gn - 1),
                                skip_group_check=True)
                        zr = pool.tile([P, GB * HID], bf16, tag="zr")
                        nc.vector.tensor_scalar(
                            out=zr[:, :gn * HID], in0=zp[:, :gn * HID],
                            scalar1=0.0, scalar2=None, op0=mybir.AluOpType.max)
                        h0w.append(nc.scalar.dma_start(
                            h_full[k][0][:][g0 * P:(g0 + gn) * P, :HID].rearrange(
                                "(t p) f -> p t f", p=P),
                            zr[:, :gn * HID].rearrange("p (t f) -> p t f", f=HID)))
                    mk = nc.sync.nop()
                    for wi in h0w:
                        add_dep_helper(mk.ins, wi.ins, sync=True,
                                       reason=f"proj-{k}-table-done")
                    pj_marker[k] = mk

            # targeted cross-layer dependencies (instead of full barriers):
            #   sbw: shard_buf writes feeding each AllGather
            #   htw: hT_loc stripe writes read by next layer's root term
            #   ag_comm: the AllGather instruction for (node type, layer)
            from concourse.tile import add_dep_helper
            sbw = {}
            htw = {}
            ag_comm = {}

            def do_layer(l):
                last = (l == NLAYERS - 1)
                # order dsts so each AllGather launches as early as possible
                # and overlaps the remaining dsts' compute:
                #   l=0: c first (its AG gates l=1 r/j); l=1: r,j first
                #   (their AGs gate l=2), c (no AG) last.
                if last:
                    dks = ["c"]
                elif l == 0:
                    dks = ["c", "r", "j"]
                else:
                    dks = ["r", "j", "c"]
                for dk in dks:
                    sbw.pop((dk, l), None)
                    ets = _DST_ETYPES[dk]
                    stripes = prep[ets[0]]["stripes"]
                    # skip shard/AG for c at layer NLAYERS-2: layer NLAYERS-1
                    # never gathers from the c table
                    need_ag = (not last) and not (dk == "c" and l == NLAYERS - 2)
                    for si, (ts, nt) in enumerate(stripes):
                        aggs = {}
                        for eti, et in enumerate(ets):
                            pr = prep[et]
                            agg = psum.tile([HID, 512], f32, space="PSUM",
                                            tag=f"agg{eti}")
                            aggs[et] = agg
                            rlist = [(i, r) for i, r in enumerate(pr["runs"])
                                     if r[0] == si]
                            n_r = len(rlist)
                            q = pr["q"]
                            cpt = q // P
                            for ri, (gi, (_, b, nt_, n_idx, n_reg)) in enumerate(rlist):
                                cofs_, w, ch = pr["comb_offsets"][gi]
                                it = gpool.tile([P, w + 2 * ch], mybir.dt.int16,
                                                tag=f"it{et}")
                                nc.sync.dma_start(
                                    it[:], comb_t[et][:, cofs_:cofs_ + w + 2 * ch])
                                dt_ = it[:, w:w + 2 * ch].bitcast(f32)
                                gt = gpool.tile([P, g_chunks[et] * FAT], bf16,
                                                tag=f"g{et}")
                                sk = _ETYPES[et][1]
                                tab = h_full[sk][l]
                                b_hi = min((b + 1) * BANK, npad[sk])
                                if "nogather" not in PROBE:
                                    gin = nc.gpsimd.dma_gather(
                                        out_ap=gt[:, :ch * FAT].rearrange(
                                            "p (c f) -> p c f", f=FAT),
                                        in_ap=tab[:][b * BANK:b_hi, :],
                                        idxs_ap=it[:, :w],
                                        num_idxs=n_idx, num_idxs_reg=nregs[n_reg],
                                        elem_size=FAT, single_packet=False,
                                        queue_num=qrot[0] % 4)
                                    if l >= 1 and (sk, l - 1) in ag_comm:
                                        add_dep_helper(
                                            gin.ins, ag_comm[(sk, l - 1)].ins,
                                            sync=True, reason="gather-after-ag")
                                    elif l == 0 and sk in pj_marker:
                                        add_dep_helper(
                                            gin.ins, pj_marker[sk].ins,
                                            sync=True, reason="gather-after-proj")
                                qrot[0] += 1
                                # one-hot per chunk: tensor_scalar with the
                                # dstl column as per-partition f32 scalar --
                                # iota/oh fp16 qualify for DVE fast modes
                                for c in range(ch):
                                    is_ovf = (c >= nt_ * cpt)
                                    wdt = nt_ * P if is_ovf else P
                                    io = iota512 if is_ovf else iota128
                                    oh = ohpool.tile([P, 512], fp16, tag="oh")
                                    nc.vector.tensor_scalar(
                                        out=oh[:, :wdt], in0=io[:, :wdt],
                                        scalar1=dt_[:, c:c + 1], scalar2=None,
                                        op0=mybir.AluOpType.is_equal)
                                    lastmm = (ri == n_r - 1 and c == ch - 1)
                                    if is_ovf:
                                        nc.tensor.matmul(
                                            out=agg[:, :wdt],
                                            lhsT=gt[:, c * FAT:c * FAT + HID],
                                            rhs=oh[:, :wdt],
                                            start=False, stop=lastmm,
                                            skip_group_check=True)
                                    else:
                                        tt = c // cpt
                                        o0, o1 = tt * P, (tt + 1) * P
                                        first = (ri == 0 and c == 0)
                                        nc.tensor.matmul(
                                            out=agg[:, o0:o1],
                                            lhsT=gt[:, c * FAT:c * FAT + HID],
                                            rhs=oh[:, :wdt],
                                            start=first, stop=lastmm,
                                            skip_group_check=True)
                        aggsb = {}
                        for et in ets:
                            a = pool.tile([HID, 512], bf16, tag=f"aggsb{et}",
                                          name=f"aggsb{et}")
                            nc.vector.tensor_copy(a[:, :nt * P],
                                                  aggs[et][:, :nt * P])
                            aggsb[et] = a
                        # root-term h^T for the whole stripe in one DMA
                        hTs = pool.tile([HID, STRIPE_T * P], bf16, tag="hTt")
                        hli = nc.scalar.dma_start(
                            hTs[:, :nt * P],
                            hT_loc[dk][l][:][:, ts * P:(ts + nt) * P])
                        if l >= 1 and (dk, l, si) in htw:
                            add_dep_helper(hli.ins, htw[(dk, l, si)].ins,
                                           sync=True, reason="hT-after-write")
                        elif l == 0:
                            for g in range(ts // GBH,
                                           (ts + nt - 1) // GBH + 1):
                                if (dk, g) in pj_htw:
                                    add_dep_helper(
                                        hli.ins, pj_htw[(dk, g)].ins,
                                        sync=True, reason="hT-after-proj")
                        zsb = pool.tile([P, STRIPE_T * HID], bf16, tag="zsb")
                        if not last:
                            ztr2 = pool.tile([HID, STRIPE_T * P], bf16, tag="ztr2")
                        for j in range(nt):
                            t = ts + j
                            zmr = psum.tile([P, 256], f32, space="PSUM",
                                            tag="zmr")
                            for ei_, et in enumerate(ets):
                                nc.tensor.matmul(
                                    out=zmr[:, ei_ * HID:(ei_ + 1) * HID],
                                    lhsT=aggsb[et][:, j * P:(j + 1) * P],
                                    rhs=wl_sb[(et, l)][:],
                                    start=(ei_ == 0), stop=False,
                                    skip_group_check=True)
                            ro = 2 * HID
                            nc.tensor.matmul(out=zmr[:, ro:ro + HID],
                                             lhsT=hTs[:, j * P:(j + 1) * P],
                                             rhs=wrc_sb[(dk, l)][:],
                                             start=False,
                                             stop=not bias_nonzero[dk],
                                             skip_group_check=True)
                            if bias_nonzero[dk]:
                                nc.tensor.matmul(out=zmr[:, ro:ro + HID],
                                                 lhsT=ones1[:],
                                                 rhs=bc_sb[(dk, l)][:],
                                                 start=False, stop=True,
                                                 skip_group_check=True)
                            # z = sum_et ivc_et * zm_et + zroot, then relu
                            zrt = pool.tile([P, HID], f32, tag="zrt")
                            nc.vector.tensor_copy(zrt[:], zmr[:, ro:ro + HID])
                            tmp = pool.tile([P, HID], f32, tag="ztmp")
                            nc.vector.scalar_tensor_tensor(
                                out=tmp[:],
                                in0=zmr[:, 0:HID],
                                scalar=ivall[ets[0]][:, t:t + 1],
                                in1=zrt[:],
                                op0=mybir.AluOpType.mult,
                                op1=mybir.AluOpType.add)
                            if len(ets) > 1:
                                nc.vector.scalar_tensor_tensor(
                                    out=tmp[:],
                                    in0=zmr[:, HID:2 * HID],
                                    scalar=ivall[ets[1]][:, t:t + 1],
                                    in1=tmp[:],
                                    op0=mybir.AluOpType.mult,
                                    op1=mybir.AluOpType.add)
                            nc.vector.tensor_scalar(
                                out=zsb[:, j * HID:(j + 1) * HID], in0=tmp[:],
                                scalar1=0.0, scalar2=None,
                                op0=mybir.AluOpType.max)
                            if not last:
                                ztp = psum.tile([HID, 512], bf16, space="PSUM",
                                                tag="ztp")
                                nc.tensor.transpose(
                                    out=ztp[:, :P],
                                    in_=zsb[:, j * HID:(j + 1) * HID],
                                    identity=identb[:])
                                nc.vector.tensor_copy(
                                    ztr2[:, j * P:(j + 1) * P], ztp[:, :P])
                        if not last:
                            htw[(dk, l + 1, si)] = nc.scalar.dma_start(
                                hT_loc[dk][l + 1][:][:, ts * P:(ts + nt) * P],
                                ztr2[:, :nt * P])
                            if need_ag:
                                wi = nc.scalar.dma_start(
                                    shard_buf[dk][l][:][ts * P:(ts + nt) * P, :HID]
                                    .rearrange("(t p) f -> p t f", p=P),
                                    zsb[:, :nt * HID].rearrange(
                                        "p (t f) -> p t f", f=HID))
                                sbw.setdefault((dk, l), []).append(wi)
                        else:
                            tmp2 = pool.tile([P, STRIPE_T * HID], f32, tag="ctmp")
                            nc.vector.tensor_tensor(
                                out=tmp2[:, :nt * HID], in0=zsb[:, :nt * HID],
                                in1=wclsr2[:, :nt * HID],
                                op=mybir.AluOpType.mult)
                            ot = pool.tile([P, STRIPE_T], f32, tag="otile")
                            nc.vector.tensor_reduce(
                                out=ot[:, :nt],
                                in_=tmp2[:, :nt * HID].rearrange(
                                    "p (t f) -> p t f", f=HID),
                                axis=mybir.AxisListType.X,
                                op=mybir.AluOpType.add)
                            if b_cls != 0.0:
                                nc.vector.tensor_scalar(
                                    out=ot[:, :nt], in0=ot[:, :nt],
                                    scalar1=b_cls, scalar2=None,
                                    op0=mybir.AluOpType.add)
                            oap = out_t[:].rearrange("(t p) o -> p t o", p=P)
                            nc.scalar.dma_start(oap[:, ts:ts + nt, 0], ot[:, :nt])
                    if need_ag and "noag" not in PROBE:
                        cc = nc.gpsimd.collective_compute(
                            "AllGather", mybir.AluOpType.bypass,
                            ins=[shard_buf[dk][l][:]],
                            outs=[ag_out[dk][l][:]],
                            replica_groups=[list(range(N_CORES))])
                        for wi in sbw.get((dk, l), []):
                            add_dep_helper(cc.ins, wi.ins, sync=True,
                                           reason="ag-after-shard-writes")
                        ag_comm[(dk, l)] = cc

            for _rep in range(reps):
                if "noproj" not in PROBE:
                    input_proj()
                for l in range(NLAYERS):
                    do_layer(l)

    nc.finalize()

    iota128_v = np.broadcast_to(np.arange(P, dtype=np.float16), (P, P)).copy()
    iota512_v = np.broadcast_to(np.arange(512, dtype=np.float16), (P, 512)).copy()
    wclsr_v = np.tile(w_cls.reshape(1, HID), (P, STRIPE_T)).astype(bf16np)

    in_maps = []
    for c in range(N_CORES):
        m = {}
        for k in tpc:
            sh = tpc[k] * P
            m[f"xT_{k}"] = xT[k]
            m[f"xo_{k}"] = np.ascontiguousarray(xT[k][:, c * sh:(c + 1) * sh])
            m[f"win_{k}"] = w_in_np[k]
        for et in _ETYPES:
            m[f"comb_{et}"] = prep[et]["comb"][c]
            m[f"ivc_{et}"] = np.ascontiguousarray(iv[et][c].T)
            m[f"wl_{et}"] = WL[et].astype(bf16np)
        for dk in _DST_ETYPES:
            m[f"wrc_{dk}"] = np.asarray(WRc[dk], np.float32).astype(bf16np)
            m[f"bc_{dk}"] = np.asarray(Bc[dk], np.float32).reshape(
                NLAYERS, 1, HID).astype(bf16np)
        m["iota128"] = iota128_v
        m["iota512"] = iota512_v
        m["wclsr"] = wclsr_v
        in_maps.append(m)

    return nc, in_maps, dict(tpc=tpc, NC=NC)


def kernel(**inputs):
    import jax
    from jax.sharding import Mesh, PartitionSpec, NamedSharding
    from jax.experimental.shard_map import shard_map
    from concourse.bass2jax import (_bass_exec_p, partition_id_tensor,
                                    install_neuronx_cc_hook)
    import concourse.mybir as mybir

    nc, in_maps, meta = build(inputs)
    tpc, NC = meta["tpc"], meta["NC"]

    install_neuronx_cc_hook()
    partition_name = nc.partition_id_tensor.name if nc.partition_id_tensor else None
    in_names, out_names, out_avals, zero_outs = [], [], [], []
    for alloc in nc.m.functions[0].allocations:
        if not isinstance(alloc, mybir.MemoryLocationSet):
            continue
        name = alloc.memorylocations[0].name
        if alloc.kind == "ExternalInput":
            if name != partition_name:
                in_names.append(name)
        elif alloc.kind == "ExternalOutput":
            out_names.append(name)
            shape = tuple(alloc.tensor_shape)
            dtype = mybir.dt.np(alloc.dtype)
            out_avals.append(jax.core.ShapedArray(shape, dtype))
            zero_outs.append(np.zeros(shape, dtype))
    n_params = len(in_names)
    all_in = list(in_names) + list(out_names)
    if partition_name is not None:
        all_in.append(partition_name)

    def _body(*args):
        operands = list(args)
        if partition_name is not None:
            operands.append(partition_id_tensor())
        outs = _bass_exec_p.bind(
            *operands, out_avals=tuple(out_avals), in_names=tuple(all_in),
            out_names=tuple(out_names), lowering_input_output_aliases=(),
            sim_require_finite=False, sim_require_nnan=False, nc=nc)
        return tuple(outs)

    devices = jax.devices()[:N_CORES]
    mesh = Mesh(np.asarray(devices), ("core",))
    specs = (PartitionSpec("core"),)
    sharded = jax.jit(
        shard_map(_body, mesh=mesh, in_specs=specs * (n_params + len(out_names)),
                  out_specs=specs * len(out_names), check_rep=False),
        keep_unused=True)
    per_core = [[np.asarray(m[n]) for n in in_names] for m in in_maps]
    concat_in = [np.concatenate([per_core[c][i] for c in range(N_CORES)], axis=0)
                 for i in range(n_params)]
    concat_zero = [np.zeros((N_CORES * z.shape[0], *z.shape[1:]), z.dtype)
                   for z in zero_outs]
    shd = NamedSharding(mesh, PartitionSpec("core"))
    dev_in = [
        jax.make_array_from_callback(a.shape, shd, lambda idx, a=a: a[idx])
        for a in concat_in + concat_zero
    ]
    outs = sharded(*dev_in)
    jax.block_until_ready(outs)
    import os as _os
    if _os.environ.get("BASS_KERNEL_TIME"):
        import time as _time
        times = []
        for _ in range(int(_os.environ.get("BASS_KERNEL_REPS", "8"))):
            t0 = _time.perf_counter()
            outs2 = sharded(*dev_in)
            jax.block_until_ready(outs2)
            times.append(_time.perf_counter() - t0)
        print(f"HW exec time: {min(times) * 1e9:.0f} ns")
        print(f"exec times (s): {[f'{t:.4f}' for t in times]}")
    oi = out_names.index("out")
    full = np.asarray(outs[oi]).reshape(N_CORES * tpc["c"] * P, 1)
    return full[:NC, :].astype(np.float32)
